# revision 14
# baseline (speedup 1.0000x reference)
"""Trainium2 Bass kernel for the batched gaussian-window attention mechanism.

Math (per batch b, timestep t):
  params = softplus(X @ W + bias)            [BT, 30]
  alpha, beta, kinc = split(params, 3)       [BT, 10] each
  kappa = prev_kappa + kinc / 25
  beta  = max(beta, 0.01)
  phi[l] = sum_k alpha_k * exp(-(kappa_k - l)^2 / beta_k)
  w[c]   = sum_l phi[l] * mask[l] * onehot[l, c]

Kernel trick: alpha*exp(-(kappa-l)^2/beta) = exp(a_j + b_j*(2d) + c*(-d^2))
with d = l - l0_j the offset from a per-block center (two 32-wide l blocks,
centers 16 and 48), mu_j = kappa - l0_j, and
  a_j = ln(alpha) - mu_j^2/beta, b_j = mu_j/beta, c = 1/beta.
The block centering bounds the coefficient magnitudes so the gaussian argument
survives reduced-precision (float32r) matmul rounding; the basis entries 2d and
-d^2 (|d|<=16) are exactly representable. The [K*L, BT] gaussian argument is
then 5 chunked matmuls coefT[50, BT] against constant basis chunks [50, 128],
one exp pass (sequence mask folded into the exp bias as -1e9), and a second
matmul against replicated one-hot rows for the output projection.

Data layout: feature-major ("transposed") on device — X arrives host-transposed
as [DIN, BT]; w leaves as [C, BT]. 8-way batch-parallel across NeuronCores.
"""

import sys

sys.path.insert(0, "/opt/trn_rl_repo")

from contextlib import ExitStack

import ml_dtypes
import numpy as np

import concourse.bass as bass
import concourse.tile as tile
from concourse import bacc, mybir
from concourse.bass_utils import run_bass_kernel_spmd

AF = mybir.ActivationFunctionType
ALU = mybir.AluOpType
F32 = mybir.dt.float32
F32R = mybir.dt.float32r
BF16 = mybir.dt.bfloat16

B, T, DIN, K, L, C = 32, 1024, 400, 10, 64, 73
NCORES = 8
BC = B // NCORES           # batches per core
BT = BC * T                # rows per core
NT = BT // 128             # 128-row tiles
NI = BT // 512             # 512-col chunks
NKL = (K * L) // 128       # 128-row chunks of the (block,k,d) axis
NCF = 50                   # coefficient rows: a0 a1 b0 b1 c (10 each)
NCFP = 64                  # padded per-tile coefficient stride
BLK = 32                   # l-block width; centers 16, 48
DCH = [(0, 128), (128, 128), (256, 128), (384, 16)]  # DIN chunks

_cache: dict = {}


def _flat_to_l(flat):
    """Row index of the (block, k, d) axis -> character position l."""
    return BLK * (flat // (K * BLK)) + flat % BLK


def _build_graph():
    if "nc" in _cache:
        return _cache["nc"]
    nc = bacc.Bacc("TRN2", target_bir_lowering=False, debug=False,
                   num_devices=NCORES)

    def din(name, shape, dt=F32):
        return nc.dram_tensor(name, shape, dt, kind="ExternalInput").ap()

    xt = din("xt", [DIN, BT], BF16)          # X^T per core (host-cast bf16)
    wmat = din("wmat", [DIN, 3 * K], BF16)
    bvec = din("bvec", [3 * K, 1])
    pk = din("pk", [128, K * NT])            # prev_kappa, tile-stacked
    mmat = din("mmat", [128, NKL * 128], BF16)  # stacked hi/lo basis chunks
    ident = din("ident", [128, 128])
    identb = din("identb", [128, 128], BF16)
    e2 = din("e2", [128, NKL * BC * C], BF16)   # masked one-hot rows
    wt_o = nc.dram_tensor("wt_o", [C, BT], F32, kind="ExternalOutput").ap()
    kp_o = nc.dram_tensor("kp_o", [128, K * NT], F32,
                          kind="ExternalOutput").ap()

    with tile.TileContext(nc) as tc, ExitStack() as ctx:
        sb = ctx.enter_context(tc.tile_pool(name="sb", bufs=1))
        ps = ctx.enter_context(tc.tile_pool(name="ps", bufs=2, space="PSUM"))
        psb = ctx.enter_context(tc.tile_pool(name="psb", bufs=2, space="PSUM"))
        psw = ctx.enter_context(tc.tile_pool(name="psw", bufs=2, space="PSUM"))

        # ---- input loads ----
        msb = sb.tile([128, NKL * 128], BF16)
        nc.sync.dma_start(msb[:], mmat)
        wsb = sb.tile([128, 4 * 30], BF16)
        for j, (o, d) in enumerate(DCH):
            nc.gpsimd.dma_start(wsb[0:d, j * 30:(j + 1) * 30], wmat[o:o + d, :])
        xsb = sb.tile([128, 4 * BT], BF16)
        for h in range(2):
            for j, (o, d) in enumerate(DCH):
                nc.gpsimd.dma_start(
                    xsb[0:d, j * BT + h * (BT // 2):j * BT + (h + 1) * (BT // 2)],
                    xt[o:o + d, h * (BT // 2):(h + 1) * (BT // 2)])
        bsb = sb.tile([3 * K, 1], F32)
        nc.sync.dma_start(bsb[:], bvec)
        pksb = sb.tile([128, K * NT], F32)
        nc.sync.dma_start(pksb[:], pk)
        idsb = sb.tile([128, 128], F32)
        nc.sync.dma_start(idsb[:], ident)
        idbf = sb.tile([128, 128], BF16)
        nc.sync.dma_start(idbf[:], identb)
        e2sb = sb.tile([128, NKL * BC * C], BF16)
        nc.sync.dma_start(e2sb[:], e2)

        # ---- HAM warmup: keep PE busy during input DMA so the clock
        # gate opens before real matmuls start (idle PE runs at 1.2 GHz) ----
        for wu in range(20):
            wps_ = psb.tile([128, 1024], F32, tag="at")
            nc.tensor.matmul(wps_[:, 0:512], msb[:, 0:128], msb[:, 0:512],
                             start=True, stop=True)

        # ---- stage A: espT = exp(W^T @ X^T + b)  (softplus finished later) ----
        spT = sb.tile([3 * K, BT], F32)
        for i in range(NI):
            pT = ps.tile([3 * K, 512], F32, tag="scratch")
            for j, (o, d) in enumerate(DCH):
                nc.tensor.matmul(
                    pT[:],
                    wsb[0:d, 30 * j:30 * j + 30],
                    xsb[0:d, j * BT + 512 * i:j * BT + 512 * i + 512],
                    start=(j == 0), stop=(j == 3))
            nc.scalar.activation(spT[:, 512 * i:512 * i + 512], pT[:],
                                 AF.Exp, bias=bsb[:, 0:1])

        # ---- stage B: transpose params^T -> tile-stacked [128, 30*NT] ----
        sp = sb.tile([128, 30 * NT], F32)
        for g in range(NT // 8):
            ptr = ps.tile([128, 240], F32, tag="scratch")
            for t8 in range(8):
                t = 8 * g + t8
                nc.tensor.transpose(ptr[:, 30 * t8:30 * t8 + 30],
                                    spT[:, 128 * t:128 * t + 128],
                                    idsb[0:30, 0:30])
            nc.vector.tensor_copy(sp[:, 240 * g:240 * g + 240], ptr[:])

        # ---- stage C: elementwise coefficient construction ----
        spv = sp[:].rearrange("p (t c) -> p t c", c=30)
        av, bev, kv = spv[:, :, 0:10], spv[:, :, 10:20], spv[:, :, 20:30]
        coef = sb.tile([128, NCFP * NT], F32)
        nc.gpsimd.memset(coef[:], 0.0)
        cfv = coef[:].rearrange("p (t c) -> p t c", c=NCFP)
        ca = [cfv[:, :, 0:10], cfv[:, :, 10:20]]
        cb = [cfv[:, :, 20:30], cfv[:, :, 30:40]]
        cc = cfv[:, :, 40:50]

        def buf10(name):
            t_ = sb.tile([128, K * NT], F32, tag=name)
            return t_, t_[:].rearrange("p (t c) -> p t c", c=10)

        lnA, lav = buf10("lnA")
        bmax, bmv = buf10("bmax")
        kap, kpv = buf10("kap")
        tmp, tmv = buf10("tmp")
        mu = []
        for j in range(2):
            m_, mv_ = buf10(f"mu{j}")
            mu.append(mv_)
        pkv = pksb[:].rearrange("p (t c) -> p t c", c=10)

        nc.scalar.activation(sp[:], sp[:], AF.Ln, bias=1.0)       # softplus
        nc.scalar.activation(lav, av, AF.Ln)                      # ln(alpha)
        nc.vector.tensor_scalar_max(bmv, bev, 0.01)               # clip beta
        nc.vector.reciprocal(cc, bmv)                             # c = 1/beta
        nc.vector.scalar_tensor_tensor(kpv, kv, 0.04, pkv,
                                       ALU.mult, ALU.add)         # kappa
        nc.sync.dma_start(kp_o, kap[:])                         # kappa out
        for j in range(2):
            l0 = float(BLK * j + BLK // 2)
            nc.vector.tensor_scalar_add(mu[j], kpv, -l0)          # mu_j
            nc.vector.tensor_mul(cb[j], mu[j], cc)                # b_j
            nc.vector.tensor_mul(tmv, mu[j], cb[j])               # mu_j^2/beta
            nc.vector.tensor_sub(ca[j], lav, tmv)                 # a_j

        # HAM bridge: PE is otherwise idle during this elementwise phase and
        # would re-throttle to 1.2 GHz (MID window ~3.4us); chew on kappa.
        for wu in range(10):
            wps_ = psb.tile([128, 1024], F32, tag="at")
            nc.tensor.matmul(wps_[:, 0:320], idsb[:], kap[:], start=True,
                             stop=True)

        # hi/lo bf16 split so two accumulating bf16 matmuls recover f32-ish
        chi = sb.tile([128, NCFP * NT], BF16)
        clo = sb.tile([128, NCFP * NT], BF16)
        nc.vector.tensor_copy(chi[:], coef[:])
        nc.vector.tensor_sub(clo[:], coef[:], chi[:])

        for wu in range(8):
            wps_ = psb.tile([128, 1024], F32, tag="at")
            nc.tensor.matmul(wps_[:, 0:512], idbf[:], chi[:, 0:512],
                             start=True, stop=True)

        # ---- stage D: transpose hi/lo into stacked coefT2 [128, BT] bf16:
        # rows 0:64 = hi coeffs (+ zero pad), rows 64:128 = lo coeffs ----
        coefT2 = sb.tile([128, BT], BF16)
        for g in range(NT // 4):
            ctr = ps.tile([128, 512], BF16, tag="scratch")
            for t4 in range(4):
                t = 4 * g + t4
                nc.tensor.transpose(ctr[0:64, 128 * t4:128 * t4 + 128],
                                    chi[:, NCFP * t:NCFP * t + NCFP], idbf[:])
                nc.tensor.transpose(ctr[64:128, 128 * t4:128 * t4 + 128],
                                    clo[:, NCFP * t:NCFP * t + NCFP], idbf[:])
            nc.vector.tensor_copy(coefT2[:, 512 * g:512 * g + 512], ctr[:])

        # ---- stages E+F interleaved per bt-chunk i ----
        # E: argT chunks = Mc^T @ coefT (hi+lo), gauss = exp(argT + maskbias)
        # F: w^T[:, i] = sum_c E2_c^T @ gauss_c, DMA out per chunk
        gauss = sb.tile([128, NKL * BT], BF16)
        wtsb = sb.tile([C, BT], F32)
        for i in range(NI):
            bi = i // (NI // BC)
            gbase = i * NKL * 512
            for cp in range((NKL + 1) // 2):      # paired psum banks for exp
                c0 = 2 * cp
                nch = min(2, NKL - c0)
                at = psb.tile([128, 1024], F32, tag="at")
                for cc_ in range(nch):
                    c = c0 + cc_
                    nc.tensor.matmul(at[:, 512 * cc_:512 * cc_ + 512],
                                     msb[:, 128 * c:128 * c + 128],
                                     coefT2[:, 512 * i:512 * i + 512],
                                     start=True, stop=True)
                nc.scalar.activation(
                    gauss[:, gbase + 512 * c0:gbase + 512 * (c0 + nch)],
                    at[:, 0:512 * nch], AF.Exp)
            wps = psw.tile([C, 512], F32, tag="wps")
            for c in range(NKL):
                e_col = (c * BC + bi) * C
                nc.tensor.matmul(wps[:], e2sb[:, e_col:e_col + C],
                                 gauss[:, gbase + 512 * c:gbase + 512 * (c + 1)],
                                 start=(c == 0), stop=(c == NKL - 1))
            nc.vector.tensor_copy(wtsb[:, 512 * i:512 * i + 512], wps[:])
            nc.gpsimd.dma_start(wt_o[:, 512 * i:512 * i + 512],
                                wtsb[:, 512 * i:512 * i + 512])

    nc.compile()
    _cache["nc"] = nc
    return nc


def _host_constants():
    if "consts" in _cache:
        return _cache["consts"]
    flat = np.arange(K * L)
    jj = flat // (K * BLK)           # l block
    kk = (flat // BLK) % K           # gaussian component
    dd = (flat % BLK) - BLK // 2     # offset from block center
    M = np.zeros((128, NKL * 128), np.float32)
    for c in range(NKL):
        for r in range(128):
            f = 128 * c + r
            col = 128 * c + r
            for off in (0, 64):                       # hi rows, lo rows
                M[off + jj[f] * 10 + kk[f], col] = 1.0        # a_j row
                M[off + 20 + jj[f] * 10 + kk[f], col] = 2.0 * dd[f]
                M[off + 40 + kk[f], col] = -float(dd[f]) ** 2
    ident = np.eye(128, dtype=np.float32)
    identb = np.eye(128).astype(ml_dtypes.bfloat16)
    lofr = _flat_to_l(flat).reshape(NKL, 128)         # l per (chunk, row)
    M = M.astype(ml_dtypes.bfloat16)
    _cache["consts"] = (M, ident, identb, lofr)
    return M, ident, identb, lofr


def kernel(inputs, prev_kappa, char_seq_one_hot, char_seq_len, W, b):
    nc = _build_graph()
    M, ident, identb, lofr = _host_constants()
    Wf = np.ascontiguousarray(W).astype(ml_dtypes.bfloat16)
    bf = np.ascontiguousarray(b, np.float32).reshape(3 * K, 1)

    in_maps = []
    for core in range(NCORES):
        bs = slice(core * BC, (core + 1) * BC)
        Xc = np.ascontiguousarray(
            inputs[bs].reshape(BT, DIN).T).astype(ml_dtypes.bfloat16)
        pkc = np.ascontiguousarray(
            prev_kappa[bs].reshape(NT, 128, K).transpose(1, 0, 2)
            .reshape(128, K * NT), np.float32)
        ohc = np.asarray(char_seq_one_hot[bs], np.float32)      # [BC, L, C]
        mk = (np.arange(L)[None, :] <
              np.asarray(char_seq_len[bs])[:, None])            # [BC, L]
        ohm = ohc * mk[:, :, None].astype(np.float32)
        # e2[r, (c*BC+b)*C : +C] = mask[b, l]*onehot[b, l(c, r), :]
        e2c = ohm[:, lofr, :]                       # [BC, NKL, 128, C]
        e2c = np.ascontiguousarray(
            e2c.transpose(2, 1, 0, 3).reshape(128, NKL * BC * C)
        ).astype(ml_dtypes.bfloat16)
        in_maps.append({
            "xt": Xc, "wmat": Wf, "bvec": bf, "pk": pkc, "mmat": M,
            "ident": ident, "identb": identb, "e2": e2c,
        })

    _cache["in_maps"] = in_maps
    res = run_bass_kernel_spmd(nc, in_maps, core_ids=list(range(NCORES)))

    w_full = np.empty((B, T, C), np.float32)
    kap_full = np.empty((B, T, K), np.float32)
    for core in range(NCORES):
        wt = np.asarray(res.results[core]["wt_o"])          # [C, BT]
        kp = np.asarray(res.results[core]["kp_o"])          # [128, K*NT]
        w_full[core * BC:(core + 1) * BC] = \
            np.ascontiguousarray(wt.T).reshape(BC, T, C)
        kapc = kp.reshape(128, NT, K).transpose(1, 0, 2).reshape(BT, K)
        kap_full[core * BC:(core + 1) * BC] = kapc.reshape(BC, T, K)
    return w_full, kap_full[..., None]


# revision 15
# speedup vs baseline: 1.2134x; 1.2134x over previous
"""Trainium2 Bass kernel for the batched gaussian-window attention mechanism.

Math (per batch b, timestep t):
  params = softplus(X @ W + bias)            [BT, 30]
  alpha, beta, kinc = split(params, 3)       [BT, 10] each
  kappa = prev_kappa + kinc / 25
  beta  = max(beta, 0.01)
  phi[l] = sum_k alpha_k * exp(-(kappa_k - l)^2 / beta_k)
  w[c]   = sum_l phi[l] * mask[l] * onehot[l, c]

Kernel trick: alpha*exp(-(kappa-l)^2/beta) = exp(a_j + b_j*(2d) + c*(-d^2))
with d = l - l0_j the offset from a per-block center (two 32-wide l blocks,
centers 16 and 48), mu_j = kappa - l0_j, and
  a_j = ln(alpha) - mu_j^2/beta, b_j = mu_j/beta, c = 1/beta.
The block centering bounds the coefficient magnitudes so the gaussian argument
survives reduced-precision (float32r) matmul rounding; the basis entries 2d and
-d^2 (|d|<=16) are exactly representable. The [K*L, BT] gaussian argument is
then 5 chunked matmuls coefT[50, BT] against constant basis chunks [50, 128],
one exp pass (sequence mask folded into the exp bias as -1e9), and a second
matmul against replicated one-hot rows for the output projection.

Data layout: feature-major ("transposed") on device — X arrives host-transposed
as [DIN, BT]; w leaves as [C, BT]. 8-way batch-parallel across NeuronCores.
"""

import sys

sys.path.insert(0, "/opt/trn_rl_repo")

from contextlib import ExitStack

import ml_dtypes
import numpy as np

import concourse.bass as bass
import concourse.tile as tile
from concourse import bacc, mybir
from concourse.bass_utils import run_bass_kernel_spmd

AF = mybir.ActivationFunctionType
ALU = mybir.AluOpType
F32 = mybir.dt.float32
F32R = mybir.dt.float32r
BF16 = mybir.dt.bfloat16

B, T, DIN, K, L, C = 32, 1024, 400, 10, 64, 73
NCORES = 8
BC = B // NCORES           # batches per core
BT = BC * T                # rows per core
NT = BT // 128             # 128-row tiles
NI = BT // 512             # 512-col chunks
NKL = (K * L) // 128       # 128-row chunks of the (block,k,d) axis
NCF = 50                   # coefficient rows: a0 a1 b0 b1 c (10 each)
NCFP = 64                  # padded per-tile coefficient stride
BLK = 32                   # l-block width; centers 16, 48
DCH = [(0, 128), (128, 128), (256, 128), (384, 16)]  # DIN chunks

_cache: dict = {}


def _flat_to_l(flat):
    """Row index of the (block, k, d) axis -> character position l."""
    return BLK * (flat // (K * BLK)) + flat % BLK


def _build_graph():
    if "nc" in _cache:
        return _cache["nc"]
    nc = bacc.Bacc("TRN2", target_bir_lowering=False, debug=False,
                   num_devices=NCORES)

    def din(name, shape, dt=F32):
        return nc.dram_tensor(name, shape, dt, kind="ExternalInput").ap()

    xt = din("xt", [DIN, BT], BF16)          # X^T per core (host-cast bf16)
    wmat = din("wmat", [DIN, 3 * K], BF16)
    bvec = din("bvec", [3 * K, 1])
    pk = din("pk", [128, K * NT])            # prev_kappa, tile-stacked
    mmat = din("mmat", [128, NKL * 128], BF16)  # stacked hi/lo basis chunks
    ident = din("ident", [128, 128])
    identb = din("identb", [128, 128], BF16)
    e2 = din("e2", [128, NKL * BC * C], BF16)   # masked one-hot rows
    wt_o = nc.dram_tensor("wt_o", [C, BT], F32, kind="ExternalOutput").ap()
    kp_o = nc.dram_tensor("kp_o", [128, K * NT], F32,
                          kind="ExternalOutput").ap()

    with tile.TileContext(nc) as tc, ExitStack() as ctx:
        sb = ctx.enter_context(tc.tile_pool(name="sb", bufs=1))
        ps = ctx.enter_context(tc.tile_pool(name="ps", bufs=2, space="PSUM"))
        psb = ctx.enter_context(tc.tile_pool(name="psb", bufs=2, space="PSUM"))
        psw = ctx.enter_context(tc.tile_pool(name="psw", bufs=2, space="PSUM"))

        # ---- input loads (xt streamed half-major to match the pipeline) ----
        msb = sb.tile([128, NKL * 128], BF16)
        nc.sync.dma_start(msb[:], mmat)
        wsb = sb.tile([128, 4 * 30], BF16)
        for j, (o, d) in enumerate(DCH):
            nc.gpsimd.dma_start(wsb[0:d, j * 30:(j + 1) * 30], wmat[o:o + d, :])
        xsb = sb.tile([128, 4 * BT], BF16)
        for h in range(2):
            for j, (o, d) in enumerate(DCH):
                nc.gpsimd.dma_start(
                    xsb[0:d, j * BT + h * (BT // 2):j * BT + (h + 1) * (BT // 2)],
                    xt[o:o + d, h * (BT // 2):(h + 1) * (BT // 2)])
        bsb = sb.tile([3 * K, 1], F32)
        nc.sync.dma_start(bsb[:], bvec)
        pksb = sb.tile([128, K * NT], F32)
        nc.sync.dma_start(pksb[:], pk)
        idsb = sb.tile([128, 128], F32)
        nc.sync.dma_start(idsb[:], ident)
        idbf = sb.tile([128, 128], BF16)
        nc.sync.dma_start(idbf[:], identb)
        e2sb = sb.tile([128, NKL * BC * C], BF16)
        nc.sync.dma_start(e2sb[:], e2)

        # ---- HAM warmup: keep PE clocked up while inputs stream in ----
        for wu in range(14):
            wps_ = psb.tile([128, 1024], F32, tag="at")
            nc.tensor.matmul(wps_[:, 0:512], msb[:, 0:128], msb[:, 0:512],
                             start=True, stop=True)

        # ---- global buffers ----
        spT = sb.tile([3 * K, BT], F32)
        sp = sb.tile([128, 30 * NT], F32)
        coef = sb.tile([128, NCFP * NT], F32)
        nc.gpsimd.memset(coef[:], 0.0)
        # chilo: per tile, cols 0:64 = hi(bf16) coeffs, 64:128 = lo residual
        chilo = sb.tile([128, 128 * NT], BF16)
        coefT2 = sb.tile([128, BT], BF16)
        gauss = sb.tile([128, NKL * BT], BF16)
        wtsb = sb.tile([C, BT], F32)
        lnA = sb.tile([128, K * NT], F32, tag="lnA")
        bmax = sb.tile([128, K * NT], F32, tag="bmax")
        kap = sb.tile([128, K * NT], F32, tag="kap")
        tmp = sb.tile([128, K * NT], F32, tag="tmp")
        mu0 = sb.tile([128, K * NT], F32, tag="mu0")
        mu1 = sb.tile([128, K * NT], F32, tag="mu1")

        spv = sp[:].rearrange("p (t c) -> p t c", c=30)
        cfv = coef[:].rearrange("p (t c) -> p t c", c=NCFP)
        clv = chilo[:].rearrange("p (t c) -> p t c", c=128)
        HT = NT // 2            # tiles per half

        def tview(buf, c):
            return buf[:].rearrange("p (t c) -> p t c", c=c)

        def stage_a(h):
            for i in range(h * NI // 2, (h + 1) * NI // 2):
                pT = ps.tile([3 * K, 512], F32, tag="scratch")
                for j, (o, d) in enumerate(DCH):
                    nc.tensor.matmul(
                        pT[:], wsb[0:d, 30 * j:30 * j + 30],
                        xsb[0:d, j * BT + 512 * i:j * BT + 512 * i + 512],
                        start=(j == 0), stop=(j == 3))
                nc.scalar.activation(spT[:, 512 * i:512 * i + 512], pT[:],
                                     AF.Exp, bias=bsb[:, 0:1])

        def stage_b(h):
            for g in range(2 * h, 2 * h + 2):
                ptr = ps.tile([128, 240], F32, tag="scratch")
                for t8 in range(8):
                    t = 8 * g + t8
                    nc.tensor.transpose(ptr[:, 30 * t8:30 * t8 + 30],
                                        spT[:, 128 * t:128 * t + 128],
                                        idsb[0:30, 0:30])
                nc.vector.tensor_copy(sp[:, 240 * g:240 * g + 240], ptr[:])

        def stage_c(h):
            ts = slice(HT * h, HT * (h + 1))
            av, bev, kv = (spv[:, ts, 0:10], spv[:, ts, 10:20],
                           spv[:, ts, 20:30])
            ca = [cfv[:, ts, 0:10], cfv[:, ts, 10:20]]
            cb = [cfv[:, ts, 20:30], cfv[:, ts, 30:40]]
            cc = cfv[:, ts, 40:50]
            lav = tview(lnA, 10)[:, ts, :]
            bmv = tview(bmax, 10)[:, ts, :]
            kpv = tview(kap, 10)[:, ts, :]
            tmv = tview(tmp, 10)[:, ts, :]
            muv = [tview(mu0, 10)[:, ts, :], tview(mu1, 10)[:, ts, :]]
            pkv = tview(pksb, 10)[:, ts, :]
            hw = 240 * HT // 8  # stacked sp cols per half
            nc.scalar.activation(sp[:, hw * h:hw * (h + 1)],
                                 sp[:, hw * h:hw * (h + 1)], AF.Ln,
                                 bias=1.0)                        # softplus
            nc.scalar.activation(lav, av, AF.Ln)                  # ln(alpha)
            nc.vector.tensor_scalar_max(bmv, bev, 0.01)
            nc.vector.reciprocal(cc, bmv)                         # 1/beta
            nc.vector.scalar_tensor_tensor(kpv, kv, 0.04, pkv,
                                           ALU.mult, ALU.add)     # kappa
            nc.sync.dma_start(kp_o[:, 10 * HT * h:10 * HT * (h + 1)],
                              kap[:, 10 * HT * h:10 * HT * (h + 1)])
            for j in range(2):
                l0 = float(BLK * j + BLK // 2)
                nc.vector.tensor_scalar_add(muv[j], kpv, -l0)
                nc.vector.tensor_mul(cb[j], muv[j], cc)           # b_j
                nc.vector.tensor_mul(tmv, muv[j], cb[j])          # mu^2/beta
                nc.vector.tensor_sub(ca[j], lav, tmv)             # a_j
            # hi/lo split into the interleaved chilo layout
            cfull = cfv[:, ts, :]
            nc.vector.tensor_copy(clv[:, ts, 0:64], cfull)
            nc.vector.tensor_sub(clv[:, ts, 64:128], cfull,
                                 clv[:, ts, 0:64])

        def stage_d(h):
            for g in range(4 * h, 4 * h + 4):
                ctr = ps.tile([128, 512], BF16, tag="scratch")
                for t4 in range(4):
                    t = 4 * g + t4
                    nc.tensor.transpose(ctr[:, 128 * t4:128 * t4 + 128],
                                        chilo[:, 128 * t:128 * t + 128],
                                        idbf[:])
                nc.vector.tensor_copy(coefT2[:, 512 * g:512 * g + 512],
                                      ctr[:])

        def stage_ef(h):
            for i in range(h * NI // 2, (h + 1) * NI // 2):
                bi = i // (NI // BC)
                gbase = i * NKL * 512
                for cp in range((NKL + 1) // 2):
                    c0 = 2 * cp
                    nch = min(2, NKL - c0)
                    at = psb.tile([128, 1024], F32, tag="at")
                    for cc_ in range(nch):
                        c = c0 + cc_
                        nc.tensor.matmul(at[:, 512 * cc_:512 * cc_ + 512],
                                         msb[:, 128 * c:128 * c + 128],
                                         coefT2[:, 512 * i:512 * i + 512],
                                         start=True, stop=True)
                    nc.scalar.activation(
                        gauss[:, gbase + 512 * c0:gbase + 512 * (c0 + nch)],
                        at[:, 0:512 * nch], AF.Exp)
                wps = psw.tile([C, 512], F32, tag="wps")
                for c in range(NKL):
                    e_col = (c * BC + bi) * C
                    nc.tensor.matmul(
                        wps[:], e2sb[:, e_col:e_col + C],
                        gauss[:, gbase + 512 * c:gbase + 512 * (c + 1)],
                        start=(c == 0), stop=(c == NKL - 1))
                nc.vector.tensor_copy(wtsb[:, 512 * i:512 * i + 512], wps[:])
                nc.gpsimd.dma_start(wt_o[:, 512 * i:512 * i + 512],
                                    wtsb[:, 512 * i:512 * i + 512])

        # ---- half-pipelined emission: PE never waits on a full global
        # elementwise phase; the other half keeps it busy (and HAM warm) ----
        stage_a(0)
        stage_b(0)
        stage_c(0)
        stage_a(1)
        stage_b(1)
        stage_d(0)
        stage_c(1)
        stage_ef(0)
        stage_d(1)
        stage_ef(1)

    nc.compile()
    _cache["nc"] = nc
    return nc


def _host_constants():
    if "consts" in _cache:
        return _cache["consts"]
    flat = np.arange(K * L)
    jj = flat // (K * BLK)           # l block
    kk = (flat // BLK) % K           # gaussian component
    dd = (flat % BLK) - BLK // 2     # offset from block center
    M = np.zeros((128, NKL * 128), np.float32)
    for c in range(NKL):
        for r in range(128):
            f = 128 * c + r
            col = 128 * c + r
            for off in (0, 64):                       # hi rows, lo rows
                M[off + jj[f] * 10 + kk[f], col] = 1.0        # a_j row
                M[off + 20 + jj[f] * 10 + kk[f], col] = 2.0 * dd[f]
                M[off + 40 + kk[f], col] = -float(dd[f]) ** 2
    ident = np.eye(128, dtype=np.float32)
    identb = np.eye(128).astype(ml_dtypes.bfloat16)
    lofr = _flat_to_l(flat).reshape(NKL, 128)         # l per (chunk, row)
    M = M.astype(ml_dtypes.bfloat16)
    _cache["consts"] = (M, ident, identb, lofr)
    return M, ident, identb, lofr


def kernel(inputs, prev_kappa, char_seq_one_hot, char_seq_len, W, b):
    nc = _build_graph()
    M, ident, identb, lofr = _host_constants()
    Wf = np.ascontiguousarray(W).astype(ml_dtypes.bfloat16)
    bf = np.ascontiguousarray(b, np.float32).reshape(3 * K, 1)

    in_maps = []
    for core in range(NCORES):
        bs = slice(core * BC, (core + 1) * BC)
        Xc = np.ascontiguousarray(
            inputs[bs].reshape(BT, DIN).T).astype(ml_dtypes.bfloat16)
        pkc = np.ascontiguousarray(
            prev_kappa[bs].reshape(NT, 128, K).transpose(1, 0, 2)
            .reshape(128, K * NT), np.float32)
        ohc = np.asarray(char_seq_one_hot[bs], np.float32)      # [BC, L, C]
        mk = (np.arange(L)[None, :] <
              np.asarray(char_seq_len[bs])[:, None])            # [BC, L]
        ohm = ohc * mk[:, :, None].astype(np.float32)
        # e2[r, (c*BC+b)*C : +C] = mask[b, l]*onehot[b, l(c, r), :]
        e2c = ohm[:, lofr, :]                       # [BC, NKL, 128, C]
        e2c = np.ascontiguousarray(
            e2c.transpose(2, 1, 0, 3).reshape(128, NKL * BC * C)
        ).astype(ml_dtypes.bfloat16)
        in_maps.append({
            "xt": Xc, "wmat": Wf, "bvec": bf, "pk": pkc, "mmat": M,
            "ident": ident, "identb": identb, "e2": e2c,
        })

    _cache["in_maps"] = in_maps
    res = run_bass_kernel_spmd(nc, in_maps, core_ids=list(range(NCORES)))

    w_full = np.empty((B, T, C), np.float32)
    kap_full = np.empty((B, T, K), np.float32)
    for core in range(NCORES):
        wt = np.asarray(res.results[core]["wt_o"])          # [C, BT]
        kp = np.asarray(res.results[core]["kp_o"])          # [128, K*NT]
        w_full[core * BC:(core + 1) * BC] = \
            np.ascontiguousarray(wt.T).reshape(BC, T, C)
        kapc = kp.reshape(128, NT, K).transpose(1, 0, 2).reshape(BT, K)
        kap_full[core * BC:(core + 1) * BC] = kapc.reshape(BC, T, K)
    return w_full, kap_full[..., None]


# revision 17
# speedup vs baseline: 1.2685x; 1.0455x over previous
"""Trainium2 Bass kernel for the batched gaussian-window attention mechanism.

Math (per batch b, timestep t):
  params = softplus(X @ W + bias)            [BT, 30]
  alpha, beta, kinc = split(params, 3)       [BT, 10] each
  kappa = prev_kappa + kinc / 25
  beta  = max(beta, 0.01)
  phi[l] = sum_k alpha_k * exp(-(kappa_k - l)^2 / beta_k)
  w[c]   = sum_l phi[l] * mask[l] * onehot[l, c]

Kernel trick: alpha*exp(-(kappa-l)^2/beta) = exp(a_j + b_j*(2d) + c*(-d^2))
with d = l - l0_j the offset from a per-block center (two 32-wide l blocks,
centers 16 and 48), mu_j = kappa - l0_j, and
  a_j = ln(alpha) - mu_j^2/beta, b_j = mu_j/beta, c = 1/beta.
The block centering bounds the coefficient magnitudes so the gaussian argument
survives reduced-precision (float32r) matmul rounding; the basis entries 2d and
-d^2 (|d|<=16) are exactly representable. The [K*L, BT] gaussian argument is
then 5 chunked matmuls coefT[50, BT] against constant basis chunks [50, 128],
one exp pass (sequence mask folded into the exp bias as -1e9), and a second
matmul against replicated one-hot rows for the output projection.

Data layout: feature-major ("transposed") on device — X arrives host-transposed
as [DIN, BT]; w leaves as [C, BT]. 8-way batch-parallel across NeuronCores.
"""

import sys

sys.path.insert(0, "/opt/trn_rl_repo")

from contextlib import ExitStack

import ml_dtypes
import numpy as np

import concourse.bass as bass
import concourse.tile as tile
from concourse import bacc, mybir
from concourse.bass_utils import run_bass_kernel_spmd

AF = mybir.ActivationFunctionType
ALU = mybir.AluOpType
F32 = mybir.dt.float32
F32R = mybir.dt.float32r
BF16 = mybir.dt.bfloat16

B, T, DIN, K, L, C = 32, 1024, 400, 10, 64, 73
NCORES = 8
BC = B // NCORES           # batches per core
BT = BC * T                # rows per core
NT = BT // 128             # 128-row tiles
NI = BT // 512             # 512-col chunks
NKL = (K * L) // 128       # 128-row chunks of the (block,k,d) axis
NCF = 50                   # coefficient rows: a0 a1 b0 b1 c (10 each)
NCFP = 64                  # padded per-tile coefficient stride
BLK = 32                   # l-block width; centers 16, 48
DCH = [(0, 128), (128, 128), (256, 128), (384, 16)]  # DIN chunks

_cache: dict = {}


def _flat_to_l(flat):
    """Row index of the (block, k, d) axis -> character position l."""
    return BLK * (flat // (K * BLK)) + flat % BLK


def _build_graph():
    if "nc" in _cache:
        return _cache["nc"]
    nc = bacc.Bacc("TRN2", target_bir_lowering=False, debug=False,
                   num_devices=NCORES)

    def din(name, shape, dt=F32):
        return nc.dram_tensor(name, shape, dt, kind="ExternalInput").ap()

    xt = din("xt", [DIN, BT], BF16)          # X^T per core (host-cast bf16)
    wmat = din("wmat", [DIN, 3 * K], BF16)
    bvec = din("bvec", [3 * K, 1])
    pk = din("pk", [128, K * NT])            # prev_kappa, tile-stacked
    mmat = din("mmat", [128, NKL * 128], BF16)  # stacked hi/lo basis chunks
    ident = din("ident", [128, 128])
    identb = din("identb", [128, 128], BF16)
    e2 = din("e2", [128, NKL * BC * C], BF16)   # masked one-hot rows
    wt_o = nc.dram_tensor("wt_o", [C, BT], F32, kind="ExternalOutput").ap()
    kp_o = nc.dram_tensor("kp_o", [128, K * NT], F32,
                          kind="ExternalOutput").ap()

    with tile.TileContext(nc) as tc, ExitStack() as ctx:
        sb = ctx.enter_context(tc.tile_pool(name="sb", bufs=1))
        ps = ctx.enter_context(tc.tile_pool(name="ps", bufs=2, space="PSUM"))
        psb = ctx.enter_context(tc.tile_pool(name="psb", bufs=2, space="PSUM"))
        psw = ctx.enter_context(tc.tile_pool(name="psw", bufs=2, space="PSUM"))

        # ---- input loads (xt streamed half-major to match the pipeline) ----
        msb = sb.tile([128, NKL * 128], BF16)
        nc.sync.dma_start(msb[:], mmat)
        wsb = sb.tile([128, 4 * 30], BF16)
        for j, (o, d) in enumerate(DCH):
            nc.gpsimd.dma_start(wsb[0:d, j * 30:(j + 1) * 30], wmat[o:o + d, :])
        xsb = sb.tile([128, 4 * BT], BF16)
        for h in range(2):
            for j, (o, d) in enumerate(DCH):
                nc.gpsimd.dma_start(
                    xsb[0:d, j * BT + h * (BT // 2):j * BT + (h + 1) * (BT // 2)],
                    xt[o:o + d, h * (BT // 2):(h + 1) * (BT // 2)])
        bsb = sb.tile([3 * K, 1], F32)
        nc.sync.dma_start(bsb[:], bvec)
        pksb = sb.tile([128, K * NT], F32)
        nc.sync.dma_start(pksb[:], pk)
        idsb = sb.tile([128, 128], F32)
        nc.sync.dma_start(idsb[:], ident)
        idbf = sb.tile([128, 128], BF16)
        nc.sync.dma_start(idbf[:], identb)
        e2sb = sb.tile([128, NKL * BC * C], BF16)
        nc.sync.dma_start(e2sb[:], e2)

        # ---- HAM warmup: keep PE clocked up while inputs stream in ----
        for wu in range(14):
            wps_ = psb.tile([128, 1024], F32, tag="at")
            nc.tensor.matmul(wps_[:, 0:512], msb[:, 0:128], msb[:, 0:512],
                             start=True, stop=True)

        # ---- global buffers ----
        spT = sb.tile([3 * K, BT], F32)
        sp = sb.tile([128, 30 * NT], F32)
        coef = sb.tile([128, NCFP * NT], F32)
        nc.gpsimd.memset(coef[:], 0.0)
        # chilo: per tile, cols 0:64 = hi(bf16) coeffs, 64:128 = lo residual
        chilo = sb.tile([128, 128 * NT], BF16)
        coefT2 = sb.tile([128, BT], BF16)
        gauss = sb.tile([128, NKL * BT], BF16)
        wtsb = sb.tile([C, BT], F32)
        lnA = sb.tile([128, K * NT], F32, tag="lnA")
        bmax = sb.tile([128, K * NT], F32, tag="bmax")
        kap = sb.tile([128, K * NT], F32, tag="kap")
        tmp = sb.tile([128, K * NT], F32, tag="tmp")
        mu0 = sb.tile([128, K * NT], F32, tag="mu0")
        mu1 = sb.tile([128, K * NT], F32, tag="mu1")

        spv = sp[:].rearrange("p (t c) -> p t c", c=30)
        cfv = coef[:].rearrange("p (t c) -> p t c", c=NCFP)
        clv = chilo[:].rearrange("p (t c) -> p t c", c=128)
        HT = NT // 2            # tiles per half

        def tview(buf, c):
            return buf[:].rearrange("p (t c) -> p t c", c=c)

        def stage_a(h):
            for i in range(h * NI // 2, (h + 1) * NI // 2):
                pT = ps.tile([3 * K, 512], F32, tag="scratch")
                for j, (o, d) in enumerate(DCH):
                    nc.tensor.matmul(
                        pT[:], wsb[0:d, 30 * j:30 * j + 30],
                        xsb[0:d, j * BT + 512 * i:j * BT + 512 * i + 512],
                        start=(j == 0), stop=(j == 3))
                nc.scalar.activation(spT[:, 512 * i:512 * i + 512], pT[:],
                                     AF.Exp, bias=bsb[:, 0:1])

        def stage_b(h):
            for g in range(2 * h, 2 * h + 2):
                ptr = ps.tile([128, 240], F32, tag="scratch")
                for t8 in range(8):
                    t = 8 * g + t8
                    nc.tensor.transpose(ptr[:, 30 * t8:30 * t8 + 30],
                                        spT[:, 128 * t:128 * t + 128],
                                        idsb[0:30, 0:30])
                nc.vector.tensor_copy(sp[:, 240 * g:240 * g + 240], ptr[:])

        def stage_c(h):
            ts = slice(HT * h, HT * (h + 1))
            av, bev, kv = (spv[:, ts, 0:10], spv[:, ts, 10:20],
                           spv[:, ts, 20:30])
            ca = [cfv[:, ts, 0:10], cfv[:, ts, 10:20]]
            cb = [cfv[:, ts, 20:30], cfv[:, ts, 30:40]]
            cc = cfv[:, ts, 40:50]
            lav = tview(lnA, 10)[:, ts, :]
            bmv = tview(bmax, 10)[:, ts, :]
            kpv = tview(kap, 10)[:, ts, :]
            tmv = tview(tmp, 10)[:, ts, :]
            muv = [tview(mu0, 10)[:, ts, :], tview(mu1, 10)[:, ts, :]]
            pkv = tview(pksb, 10)[:, ts, :]
            hw = 240 * HT // 8  # stacked sp cols per half
            nc.scalar.activation(sp[:, hw * h:hw * (h + 1)],
                                 sp[:, hw * h:hw * (h + 1)], AF.Ln,
                                 bias=1.0)                        # softplus
            nc.scalar.activation(lav, av, AF.Ln)                  # ln(alpha)
            nc.vector.tensor_scalar_max(bmv, bev, 0.01)
            nc.vector.reciprocal(cc, bmv)                         # 1/beta
            nc.vector.scalar_tensor_tensor(kpv, kv, 0.04, pkv,
                                           ALU.mult, ALU.add)     # kappa
            nc.sync.dma_start(kp_o[:, 10 * HT * h:10 * HT * (h + 1)],
                              kap[:, 10 * HT * h:10 * HT * (h + 1)])
            for j in range(2):
                l0 = float(BLK * j + BLK // 2)
                nc.vector.tensor_scalar_add(muv[j], kpv, -l0)
                nc.vector.tensor_mul(cb[j], muv[j], cc)           # b_j
                nc.vector.tensor_mul(tmv, muv[j], cb[j])          # mu^2/beta
                nc.vector.tensor_sub(ca[j], lav, tmv)             # a_j
            # hi/lo split into the interleaved chilo layout
            cfull = cfv[:, ts, :]
            nc.vector.tensor_copy(clv[:, ts, 0:64], cfull)
            nc.vector.tensor_sub(clv[:, ts, 64:128], cfull,
                                 clv[:, ts, 0:64])

        def stage_d(h):
            for g in range(4 * h, 4 * h + 4):
                ctr = ps.tile([128, 512], BF16, tag="scratch")
                for t4 in range(4):
                    t = 4 * g + t4
                    nc.tensor.transpose(ctr[:, 128 * t4:128 * t4 + 128],
                                        chilo[:, 128 * t:128 * t + 128],
                                        idbf[:])
                nc.vector.tensor_copy(coefT2[:, 512 * g:512 * g + 512],
                                      ctr[:])

        def stage_ef(h):
            for i in range(h * NI // 2, (h + 1) * NI // 2):
                bi = i // (NI // BC)
                gbase = i * NKL * 512
                for cp in range((NKL + 1) // 2):
                    c0 = 2 * cp
                    nch = min(2, NKL - c0)
                    at = psb.tile([128, 1024], F32, tag="at")
                    for cc_ in range(nch):
                        c = c0 + cc_
                        nc.tensor.matmul(at[:, 512 * cc_:512 * cc_ + 512],
                                         msb[:, 128 * c:128 * c + 128],
                                         coefT2[:, 512 * i:512 * i + 512],
                                         start=True, stop=True)
                    nc.scalar.activation(
                        gauss[:, gbase + 512 * c0:gbase + 512 * (c0 + nch)],
                        at[:, 0:512 * nch], AF.Exp)
                wps = psw.tile([C, 512], F32, tag="wps")
                for c in range(NKL):
                    e_col = (c * BC + bi) * C
                    nc.tensor.matmul(
                        wps[:], e2sb[:, e_col:e_col + C],
                        gauss[:, gbase + 512 * c:gbase + 512 * (c + 1)],
                        start=(c == 0), stop=(c == NKL - 1))
                nc.vector.tensor_copy(wtsb[:, 512 * i:512 * i + 512], wps[:])
                nc.gpsimd.dma_start(wt_o[:, 512 * i:512 * i + 512],
                                    wtsb[:, 512 * i:512 * i + 512])

        # ---- half-pipelined emission: PE never waits on a full global
        # elementwise phase; the other half keeps it busy (and HAM warm) ----
        stage_a(0)
        stage_b(0)
        stage_c(0)
        stage_a(1)
        stage_b(1)
        stage_d(0)
        stage_c(1)
        stage_ef(0)
        stage_d(1)
        stage_ef(1)

    nc.compile()
    _cache["nc"] = nc
    return nc


def _host_constants():
    if "consts" in _cache:
        return _cache["consts"]
    flat = np.arange(K * L)
    jj = flat // (K * BLK)           # l block
    kk = (flat // BLK) % K           # gaussian component
    dd = (flat % BLK) - BLK // 2     # offset from block center
    M = np.zeros((128, NKL * 128), np.float32)
    for c in range(NKL):
        for r in range(128):
            f = 128 * c + r
            col = 128 * c + r
            for off in (0, 64):                       # hi rows, lo rows
                M[off + jj[f] * 10 + kk[f], col] = 1.0        # a_j row
                M[off + 20 + jj[f] * 10 + kk[f], col] = 2.0 * dd[f]
                M[off + 40 + kk[f], col] = -float(dd[f]) ** 2
    ident = np.eye(128, dtype=np.float32)
    identb = np.eye(128).astype(ml_dtypes.bfloat16)
    lofr = _flat_to_l(flat).reshape(NKL, 128)         # l per (chunk, row)
    M = M.astype(ml_dtypes.bfloat16)
    _cache["consts"] = (M, ident, identb, lofr)
    return M, ident, identb, lofr


def kernel(inputs, prev_kappa, char_seq_one_hot, char_seq_len, W, b):
    nc = _build_graph()
    M, ident, identb, lofr = _host_constants()
    Wf = np.ascontiguousarray(W).astype(ml_dtypes.bfloat16)
    bf = np.ascontiguousarray(b, np.float32).reshape(3 * K, 1)

    in_maps = []
    for core in range(NCORES):
        bs = slice(core * BC, (core + 1) * BC)
        Xc = np.ascontiguousarray(
            inputs[bs].reshape(BT, DIN).T).astype(ml_dtypes.bfloat16)
        pkc = np.ascontiguousarray(
            prev_kappa[bs].reshape(NT, 128, K).transpose(1, 0, 2)
            .reshape(128, K * NT), np.float32)
        ohc = np.asarray(char_seq_one_hot[bs], np.float32)      # [BC, L, C]
        mk = (np.arange(L)[None, :] <
              np.asarray(char_seq_len[bs])[:, None])            # [BC, L]
        ohm = ohc * mk[:, :, None].astype(np.float32)
        # e2[r, (c*BC+b)*C : +C] = mask[b, l]*onehot[b, l(c, r), :]
        e2c = ohm[:, lofr, :]                       # [BC, NKL, 128, C]
        e2c = np.ascontiguousarray(
            e2c.transpose(2, 1, 0, 3).reshape(128, NKL * BC * C)
        ).astype(ml_dtypes.bfloat16)
        in_maps.append({
            "xt": Xc, "wmat": Wf, "bvec": bf, "pk": pkc, "mmat": M,
            "ident": ident, "identb": identb, "e2": e2c,
        })

    _cache["in_maps"] = in_maps
    res = run_bass_kernel_spmd(nc, in_maps, core_ids=list(range(NCORES)))

    w_full = np.empty((B, T, C), np.float32)
    kap_full = np.empty((B, T, K), np.float32)
    for core in range(NCORES):
        wt = np.asarray(res.results[core]["wt_o"])          # [C, BT]
        kp = np.asarray(res.results[core]["kp_o"])          # [128, K*NT]
        w_full[core * BC:(core + 1) * BC] = \
            np.ascontiguousarray(wt.T).reshape(BC, T, C)
        kapc = kp.reshape(128, NT, K).transpose(1, 0, 2).reshape(BT, K)
        kap_full[core * BC:(core + 1) * BC] = kapc.reshape(BC, T, K)
    return w_full, kap_full[..., None]
